# revision 35
# baseline (speedup 1.0000x reference)
"""Trainium2 Bass kernel for BHS_GCN: 2x GCNConv + dueling value/advantage heads.

Strategy (8 NeuronCores, single NEFF launch, bf16 dataflow / fp32 PSUM):
  - GCN phase batch-parallel: each core owns B_LOC=4 full graphs.
    Message passing = bulk dma_gather of source-node rows + PE one-hot
    scatter-matmuls into PSUM (edges pre-sorted/packed by dst on host).
    The one-hot S matrices are built once in SBUF and reused by both layers.
  - Layer-2 scatter is flipped per batch (msg_b.T @ S) so agg2 lands
    feature-major in PSUM directly - no PE transposes before the AllToAll.
  - AllToAll (bf16, 4 chunks) reshards agg2 to node-parallel: each core gets
    its 512-node slice for all 32 batches, so each core reads only its 1/8
    slice of advW/v1W (the dominating memory traffic is read once machine-wide).
  - Head contraction is flipped: the h2 block [128,32] is the stationary PE
    operand and the 76 head-weight columns stream - avoids the per-tile
    128-col weight-load bottleneck and any padding of the weight DMA.
  - Head-weight slabs stream on the Activation DMA queue (first 3 hoisted to
    program start so they prefetch during the GCN phases).
  - AllReduce of [32,76] partial head sums; the tiny val-MLP and dueling
    combine run redundantly on every core; host takes core 0's output.
"""

import os
import sys

sys.path.insert(0, "/opt/trn_rl_repo")

import numpy as np
import ml_dtypes

BF16 = np.dtype(ml_dtypes.bfloat16)

# ---------------- problem constants (hardcoded per contract) ----------------
B, N, F_IN, E = 32, 4096, 16, 16384
NC_CORES = 8
B_LOC = B // NC_CORES            # 4
NSLICE = N // NC_CORES           # 512 nodes per core for head phase
F1, F2 = 128, 256
P = 128
NTILES = N // P                  # 32 node tiles
BF1 = B_LOC * F_IN               # 64   (mp1 row width)
BFH = B_LOC * F1                 # 512  (H1 row width = mp2 gather width)
KTOT = NSLICE * F2               # 131072 contraction rows per core
KT = KTOT // P                   # 1024 K-tiles for head matmul
HW_W = 12 + 64                   # 76 head outputs (v1 | adv)
NB_H = 64                        # nodes per W2/head block
NBLK = NSLICE // NB_H            # 8 head blocks
HW_PRE = 3                       # head-weight slabs prefetched at t=0
A2A_CHUNKS = int(os.environ.get("GCN_A2A_CHUNKS", "4"))  # 1, 2, or 4


def _pack_edges(edge_index, edge_weight):
    """Sort edges by dst, pack into 128-edge chunks such that every chunk's
    dsts fall in one 128-node tile. The 128 self-loops of each tile form the
    tile's LAST chunk (srcs = the tile's own node range, ascending) so its
    "gather" is a contiguous DMA, not descriptor-based. Returns device
    tables."""
    src = np.asarray(edge_index[0], np.int64)
    dst = np.asarray(edge_index[1], np.int64)
    ew = np.asarray(edge_weight, np.float32)

    deg = np.zeros(N, np.float32)
    np.add.at(deg, dst, ew)
    deg += 1.0
    dinv = (1.0 / np.sqrt(deg)).astype(np.float32)

    src_a, dst_a = src, dst
    nrm_a = (ew * dinv[src] * dinv[dst]).astype(np.float32)
    order = np.argsort(dst_a, kind="stable")
    src_a, dst_a, nrm_a = src_a[order], dst_a[order], nrm_a[order]

    src_pk, nrm_pk, off_pk = [], [], []
    chunk_tile = []
    tile_splits = {}   # quarter-0 tiles: number of chunks with src < N/2
    for t in range(NTILES):
        sel = (dst_a >= t * P) & (dst_a < (t + 1) * P)
        s, d, w = src_a[sel], dst_a[sel], nrm_a[sel]
        # sort by src for HBM locality within the gather
        so = np.argsort(s, kind="stable")
        s, d, w = s[so], d[so], w[so]
        if t % 4 == 0:
            # split by source half so the first-half gathers can start once
            # mp1 has written H1 rows [0, N/2)
            groups = [s < N // 2, s >= N // 2]
        else:
            groups = [np.ones(len(s), bool)]
        nch_per = []
        for gsel in groups:
            sg, dg, wg = s[gsel], d[gsel], w[gsel]
            cnt = len(sg)
            nch = max(1, (cnt + P - 1) // P) if (cnt or len(groups) == 1) else 0
            pad = nch * P - cnt
            src_pk.append(np.concatenate([sg, np.zeros(pad, np.int64)]))
            nrm_pk.append(np.concatenate([wg, np.zeros(pad, np.float32)]))
            off_pk.append(np.concatenate([dg - t * P, np.zeros(pad, np.int64)]))
            chunk_tile.extend([t] * nch)
            nch_per.append(nch)
        if t % 4 == 0:
            tile_splits[t] = nch_per[0]
        # self-loop chunk: src=dst=n for the tile's 128 nodes, weight 1/deg
        rng = np.arange(t * P, (t + 1) * P, dtype=np.int64)
        src_pk.append(rng)
        nrm_pk.append((dinv[rng] * dinv[rng]).astype(np.float32))
        off_pk.append(rng - t * P)
        chunk_tile.append(t)

    src_pk = np.concatenate(src_pk)
    nrm_pk = np.concatenate(nrm_pk)
    off_pk = np.concatenate(off_pk)
    e_pad = len(src_pk)
    nchunk = e_pad // P
    assert nchunk == len(chunk_tile)

    # dma_gather index table: logical idx i lives at [i % 16, i // 16]
    gidx = np.zeros((P, e_pad // 16), np.int16)
    for p16 in range(16):
        gidx[p16, :] = src_pk[p16::16].astype(np.int16)
    gidx = np.tile(gidx[:16], (8, 1))  # replicate over all 128 partitions

    # per-chunk column tables: [p, c] = value of edge c*128+p
    nrm_t = nrm_pk.reshape(nchunk, P).T.copy()          # [128, nchunk] f32
    off_t = off_pk.reshape(nchunk, P).T.astype(np.float32).copy()
    global TILE_SPLITS
    TILE_SPLITS = tile_splits
    return gidx, nrm_t, off_t, chunk_tile, nchunk


TILE_SPLITS = {}


def _prep_host(inputs):
    """All host-side numpy preprocessing: edge packing, weight layout, batch shard."""
    x = np.asarray(inputs["x"], np.float32)
    gidx, nrm_t, off_t, chunk_tile, nchunk = _pack_edges(
        inputs["edge_index"], inputs["edge_weight"]
    )

    W1 = np.asarray(inputs["W1"], np.float32)      # [16,128]
    b1 = np.asarray(inputs["b1"], np.float32)      # [128]
    W2 = np.asarray(inputs["W2"], np.float32)      # [128,256]
    b2 = np.asarray(inputs["b2"], np.float32)      # [256]
    advW = np.asarray(inputs["advW"], np.float32)  # [N*256, 12]
    advb = np.asarray(inputs["advb"], np.float32)
    v1W = np.asarray(inputs["v1W"], np.float32)    # [N*256, 64]
    v1b = np.asarray(inputs["v1b"], np.float32)
    v2W = np.asarray(inputs["v2W"], np.float32)
    v2b = np.asarray(inputs["v2b"], np.float32)
    v3W = np.asarray(inputs["v3W"], np.float32)
    v3b = np.asarray(inputs["v3b"], np.float32)

    # W1 block-diagonal over the 4 local batches, plus a bias row driven by
    # a constant-1 row appended to aggT on device: [65, 512]
    w1bd = np.zeros((BF1 + 1, B_LOC * F1), np.float32)
    for b in range(B_LOC):
        w1bd[b * F_IN:(b + 1) * F_IN, b * F1:(b + 1) * F1] = W1
    w1bd[BF1, :] = np.tile(b1, B_LOC)

    # dueling combine matrix (adv part): out = C.T @ adv + val
    C = np.zeros((12, 12), np.float32)
    for h in range(3):
        for a in range(4):
            i = h * 4 + a
            C[i, i] += 1.0
            for a2 in range(4):
                C[h * 4 + a2, i] -= 0.25

    shared = {
        "gidx": gidx,
        "nrm_t": nrm_t.copy(),
        "off_t": off_t.copy(),
        "w1bd": w1bd.astype(BF16),
        "w2": W2.astype(BF16),
        "b2c": b2[:, None].copy(),                  # [256,1]
        "advb_c": advb[:, None].copy(),             # [12,1]
        "v1b_c": v1b[:, None].copy(),               # [64,1]
        "v2w": v2W.copy(),                          # [64,64]
        "v2b_c": v2b[:, None].copy(),               # [64,1]
        "v3w": v3W.copy(),                          # [64,1]
        "v3b_c": v3b[None, :].copy(),               # [1,1]
        "cmat": C,
    }

    per_core = []
    for j in range(NC_CORES):
        # x batch-shard, node-major rows [N, b, f] -> [N, 64] bf16, padded to
        # 128 cols so the gather's 256B row-granularity constraint holds
        x_loc = np.zeros((N, 2 * BF1), BF16)
        x_loc[:, :BF1] = (x[j * B_LOC:(j + 1) * B_LOC].transpose(1, 0, 2)
                          .reshape(N, BF1).astype(BF16))
        # head weights: rows for this core's node slice, pre-tiled to
        # [128, KT*76]: col block g holds K-tile g = rows [128g, 128g+128)
        r0 = j * KTOT
        aw = advW[r0:r0 + KTOT].reshape(KT, P, 12)
        vw = v1W[r0:r0 + KTOT].reshape(KT, P, 64)
        # v1 first (cols 0:64), adv second (64:76): after the final transpose
        # these become partition slices starting at multiples of 32.
        hw = np.concatenate([vw, aw], axis=2)        # [KT, 128, 76]
        hw_t = hw.transpose(1, 0, 2).reshape(P, KT * HW_W).astype(BF16)
        per_core.append({"x_loc": x_loc.copy(), "headw_t": hw_t.copy()})

    return shared, per_core, chunk_tile, nchunk


# ---------------- device program ----------------

def build_program(nc, tc, chunk_tile, nchunk, io, collectives=True, phases=(1, 1, 1), repeat=1,
                  a2a_chunks=None, ar_local=False, ar_mode="ag", exp=frozenset()):
    """Emit the Tile program. io: dict of name -> DRAM AP."""
    import concourse.bass as bass
    import concourse.mybir as mybir
    import concourse.tile as tile
    from concourse.masks import make_identity

    f32 = mybir.dt.float32
    bf16 = mybir.dt.bfloat16
    i16 = mybir.dt.int16
    i32 = mybir.dt.int32
    AF = mybir.ActivationFunctionType
    OP = mybir.AluOpType

    e_pad = nchunk * P
    # chunks belonging to each node tile (contiguous ranges)
    tile_chunks = [[] for _ in range(NTILES)]
    for c, t in enumerate(chunk_tile):
        tile_chunks[t].append(c)

    from contextlib import ExitStack
    with ExitStack() as ctx:
        const = ctx.enter_context(tc.tile_pool(name="const", bufs=1))
        sb = ctx.enter_context(tc.tile_pool(name="sb", bufs=3))
        sb_msg = ctx.enter_context(tc.tile_pool(name="msg", bufs=4))
        sb_msgA = ctx.enter_context(tc.tile_pool(name="msgA", bufs=6))
        sb_rhs = ctx.enter_context(tc.tile_pool(name="rhs", bufs=3))
        sb_h2 = ctx.enter_context(tc.tile_pool(name="h2", bufs=4))
        sb_hw = ctx.enter_context(tc.tile_pool(name="sbhw", bufs=HW_PRE))
        # PSUM is 8 banks of 2KB; matmul start=True zeroes a whole bank, so
        # every accumulator needs its own bank: 1 agg1 + 2 agg2 + 3 work + 1 head
        ps_a1 = ctx.enter_context(tc.tile_pool(name="ps_a1", bufs=1, space="PSUM"))
        ps_a2 = ctx.enter_context(tc.tile_pool(name="ps_a2", bufs=2, space="PSUM"))
        ps_t = ctx.enter_context(tc.tile_pool(name="ps_t", bufs=2, space="PSUM"))
        ps_w = ctx.enter_context(tc.tile_pool(name="ps_w", bufs=2, space="PSUM"))
        ps_head = ctx.enter_context(tc.tile_pool(name="ps_head", bufs=1, space="PSUM"))
        dram = ctx.enter_context(tc.tile_pool(name="dram", bufs=1, space="DRAM"))
        # ---- constants into SBUF
        ident = const.tile([P, P], f32)
        make_identity(nc, ident[:])
        ident_bf = const.tile([P, P], bf16)
        nc.vector.tensor_copy(ident_bf[:], ident[:])
        iota_i = const.tile([P, P], i32)
        nc.gpsimd.iota(iota_i[:], pattern=[[1, P]], base=0, channel_multiplier=0)
        iota_f = const.tile([P, P], f32)
        nc.vector.tensor_copy(iota_f[:], iota_i[:])
        ones1 = const.tile([1, P], f32)
        nc.vector.memset(ones1[:], 1.0)
        zcol = const.tile([P, 1], f32)
        nc.vector.memset(zcol[:], 0.0)

        gidx_sb = const.tile([P, e_pad // 16], i16)
        nc.sync.dma_start(gidx_sb[:], io["gidx"][:, :])
        nrm_sb = const.tile([P, nchunk], f32)
        nc.sync.dma_start(nrm_sb[:], io["nrm_t"][:, :])
        off_sb = const.tile([P, nchunk], f32)
        nc.sync.dma_start(off_sb[:], io["off_t"][:, :])

        w1bd_sb = const.tile([BF1 + 1, B_LOC * F1], bf16)
        nc.sync.dma_start(w1bd_sb[:], io["w1bd"][:, :])
        w2_sb = const.tile([P, F2], bf16)
        nc.sync.dma_start(w2_sb[:], io["w2"][:, :])
        # b2 [256,1] -> two [128,1] sbuf column stacks
        b2a = const.tile([P, 2], f32)
        nc.sync.dma_start(b2a[:, 0:1], io["b2c"][0:P, :])
        nc.sync.dma_start(b2a[:, 1:2], io["b2c"][P:F2, :])
        advb_sb = const.tile([12, 1], f32)
        nc.sync.dma_start(advb_sb[:], io["advb_c"][:, :])
        v1b_sb = const.tile([64, 1], f32)
        nc.sync.dma_start(v1b_sb[:], io["v1b_c"][:, :])
        v2w_sb = const.tile([64, 64], f32)
        nc.sync.dma_start(v2w_sb[:], io["v2w"][:, :])
        v2b_sb = const.tile([64, 1], f32)
        nc.sync.dma_start(v2b_sb[:], io["v2b_c"][:, :])
        v3w_sb = const.tile([64, 1], f32)
        nc.sync.dma_start(v3w_sb[:], io["v3w"][:, :])
        v3b_sb = const.tile([1, 1], f32)
        nc.sync.dma_start(v3b_sb[:], io["v3b_c"][:, :])
        cmat_sb = const.tile([12, 12], f32)
        nc.sync.dma_start(cmat_sb[:], io["cmat"][:, :])

        # one-hot scatter matrices, shared by both layers:
        # S[c][e, n] = (iota[n] == dstoff[e]) * norm[e], built per tile
        # inside mp1 (so DVE interleaves with the mp1 pipeline).
        s_all = const.tile([P, nchunk * P], bf16)
        s_built = [False] * NTILES

        def build_s(t):
            if s_built[t]:
                return
            s_built[t] = True
            for c in tile_chunks[t]:
                nc.vector.tensor_scalar(
                    out=s_all[:, c * P:(c + 1) * P], in0=iota_f[:],
                    scalar1=off_sb[:, c:c + 1], scalar2=nrm_sb[:, c:c + 1],
                    op0=OP.is_equal, op1=OP.mult,
                )

        # scratch DRAM, double-buffered across repeat bodies so consecutive
        # bodies in the timing NEFF don't serialize on WAR hazards
        h1_bufs = [dram.tile([N, BFH], bf16, name=f"h1d{r}") for r in range(2)]
        # agg2 payload dst-major: [dst core, quarter-in-chunk, n, b, f]. The
        # AllToAll exchanges along the dst dim; the head side flips (n b) x f
        # to feature-major with transposing rhs DMAs.
        NCH = a2a_chunks or A2A_CHUNKS
        if NCH == 3:
            # asymmetric schedule: merging the middle quarters saves one
            # ~15us collective control overhead while keeping the early
            # first-chunk start and the small last chunk
            CH_SCHED = [(0, 1), (1, 2), (3, 1)]
        else:
            QPC = 4 // NCH
            CH_SCHED = [(c * QPC, QPC) for c in range(NCH)]
        q_to_chunk = {q0 + i: (c, i)
                      for c, (q0, nq) in enumerate(CH_SCHED)
                      for i in range(nq)}
        a2a_in_bufs = [[dram.tile([NC_CORES, nq, P, B_LOC, F1], bf16,
                                  name=f"a2ai{c}r{r}")
                        for c, (q0, nq) in enumerate(CH_SCHED)]
                       for r in range(2)]
        a2a_out_bufs = [[dram.tile([NC_CORES, nq, P, B_LOC, F1], bf16,
                                   name=f"a2ao{c}r{r}")
                         for c, (q0, nq) in enumerate(CH_SCHED)]
                        for r in range(2)]
        ar_in_bufs = [dram.tile([HW_W, B], f32, name=f"ari{r}")
                      for r in range(2)]
        ar_out_bufs = [dram.tile([HW_W, B], f32, name=f"aro{r}")
                       for r in range(2)]


        gq_sp = "gq_sp" not in exp          # single_packet (default True)
        gq_nq = getattr(nc, "num_swdge_queues", 1)

        def gather(dst_ap, src_ap, c0, nch0, elem, seq=False, q=0):
            nidx = nch0 * P
            if seq:
                # experiment: contiguous read of the same byte count (rows 0..)
                nc.sync.dma_start(
                    dst_ap,
                    src_ap[0:P * nch0, :elem].rearrange(
                        "(c p) e -> p c e", p=P),
                )
                return
            nc.gpsimd.dma_gather(
                out_ap=dst_ap,
                in_ap=src_ap,
                idxs_ap=gidx_sb[:, c0 * 8:(c0 + nch0) * 8],
                num_idxs=nidx,
                num_idxs_reg=nidx,
                elem_size=elem,
                single_packet=gq_sp,
                queue_num=q % gq_nq,
            )

        for _rep in range(repeat):
            h1_dram = h1_bufs[_rep % 2]
            a2a_in = a2a_in_bufs[_rep % 2]
            a2a_out = a2a_out_bufs[_rep % 2]
            ar_in = ar_in_bufs[_rep % 2]
            ar_out = ar_out_bufs[_rep % 2]
            # ---- head-weight slab prefetch: first HW_PRE slabs issued on the
            # Activation DMA queue before anything else so they stream during
            # the GCN phases (no deps -> they start at t=0).
            slab_w = NB_H * 2 * HW_W                 # cols per 64-node slab
            hw_tiles = {}
            if phases[2]:
                for nb in range(min(HW_PRE, NBLK)):
                    hw_sb = sb_hw.tile([P, slab_w], bf16, tag="hwslab")
                    nc.scalar.dma_start(
                        hw_sb[:],
                        io["headw_t"][:, nb * slab_w:(nb + 1) * slab_w])
                    hw_tiles[nb] = hw_sb

            # ================= mp1 + L1 feature matmul =================
            if phases[0]:
                for t in range(NTILES):
                    cs = tile_chunks[t]
                    nch0 = len(cs)
                    build_s(t)
                    XW = 2 * BF1                         # padded x row width
                    nreg = nch0 - 1                      # last chunk = self loops
                    msg = sb_msg.tile([P, nch0 * XW], bf16, tag="msg")
                    gather(msg[:, :nreg * XW].rearrange("p (c e) -> p c e", e=XW),
                           io["x_loc"][:, :], cs[0], nreg, XW,
                           seq="mp1_seqgather" in exp, q=t)
                    # self-loop chunk: the tile's own x rows, contiguous
                    nc.sync.dma_start(msg[:, nreg * XW:nreg * XW + BF1],
                                      io["x_loc"][t * P:(t + 1) * P, 0:BF1])
                    agg = ps_a1.tile([P, BF1], f32, tag="agg1")
                    cs_eff = cs[:1] if "mp1_1mm" in exp else cs
                    for i, c in enumerate(cs_eff):
                        nc.tensor.matmul(
                            agg[:],
                            lhsT=s_all[:, c * P:(c + 1) * P],
                            rhs=msg[:, i * XW:i * XW + BF1],
                            start=(i == 0),
                            stop=(i == len(cs_eff) - 1),
                        )
                    # transpose agg [128n, 64] -> aggT [64, 128n] (bf16)
                    agg_sb = sb.tile([P, BF1], bf16, tag="agg1sb")
                    nc.vector.tensor_copy(agg_sb[:], agg[:])
                    psT = ps_t.tile([BF1, P], bf16, tag="work")
                    nc.tensor.transpose(psT[:], agg_sb[:], ident_bf[:])
                    aggT = sb.tile([BF1 + 1, P], bf16, tag="aggT1")
                    nc.vector.tensor_copy(aggT[0:BF1, :], psT[:])
                    nc.vector.memset(aggT[BF1:BF1 + 1, :], 1.0)
                    # H1[t] = relu(aggT_aug.T @ w1bd_aug)  (last row carries b1)
                    psH = ps_t.tile([P, BFH], f32, tag="work")
                    nc.tensor.matmul(psH[:], lhsT=aggT[:], rhs=w1bd_sb[:],
                                     start=True, stop=True)
                    h1sb = sb.tile([P, BFH], bf16, tag="h1sb")
                    nc.scalar.activation(h1sb[:], psH[:], AF.Relu)
                    nc.sync.dma_start(h1_dram[t * P:(t + 1) * P, :], h1sb[:])

            # ---- AllToAll for chunk c (quarters [q0, q0+nq))
            def a2a_chunk(c):
                if collectives:
                    nc.gpsimd.collective_compute(
                        "AllToAll",
                        mybir.AluOpType.bypass,
                        replica_groups=[list(range(NC_CORES))],
                        ins=[a2a_in[c][:].opt()],
                        outs=[a2a_out[c][:].opt()],
                    )
                else:
                    nc.sync.dma_start(
                        a2a_out[c][:].rearrange("s q n b f -> s q n (b f)"),
                        a2a_in[c][:].rearrange("s q n b f -> s q n (b f)"),
                    )

            # ====== head block (W2 + head partials), node-parallel ======
            SB_COLS = NB_H * B_LOC                       # 256 cols per src core
            ps_hd = ps_head.tile([HW_W, B], f32)         # [76, 32b]
            nblocks = NBLK if phases[2] else 0
            head_state = {"first": True}
            if not phases[2]:
                nc.vector.memset(ps_hd[:], 0.0)

            def head_block(nb, fire_a2a=False):
                q, half = nb // 2, nb % 2
                if fire_a2a and half == 0 and q_to_chunk[q][1] == 0:
                    a2a_chunk(q_to_chunk[q][0])
                hw_sb = hw_tiles.pop(nb, None)
                if hw_sb is None:
                    hw_sb = sb_hw.tile([P, slab_w], bf16, tag="hwslab")
                    nc.scalar.dma_start(
                        hw_sb[:],
                        io["headw_t"][:, nb * slab_w:(nb + 1) * slab_w])
                # stage rhs [128 fin, (s, n, b)] on the Activation DMA queue so
                # a collective-wait here can't head-of-line block mp2's sync
                # queue writes
                rhs_sb = sb_rhs.tile([P, NC_CORES * SB_COLS], bf16, tag="w2rhs")
                for s in range(NC_CORES):
                    # transposing DMA: [64n, 4b, 128f] dst-major payload lands
                    # as [128 f, (n b)] feature-major in SBUF via the xbar
                    nc.scalar.dma_start(
                        rhs_sb[:, s * SB_COLS:(s + 1) * SB_COLS],
                        a2a_out[q_to_chunk[q][0]][s, q_to_chunk[q][1],
                                                  half * NB_H:(half + 1) * NB_H,
                                                  :, :]
                        .rearrange("n b f -> (n b) f"),
                        transpose=True,
                    )
                # h2sb is node-major (n, s, b) so each node's 32 batch-cols
                # are contiguous (matmul operand APs need one free dim); the
                # relu copy out of PSUM does the (s,n,b)->(n,s,b) reorder.
                h2 = []
                for fh in range(2):
                    h2sb = sb_h2.tile([P, NC_CORES * SB_COLS], bf16, tag="h2sb")
                    h2v = h2sb[:].rearrange(
                        "p (n s b) -> p n s b", s=NC_CORES, n=NB_H, b=B_LOC)
                    for qq in range(4):  # free split: 512-col matmuls
                        sl = slice(qq * 512, (qq + 1) * 512)
                        psW = ps_w.tile([P, 512], f32, tag="w2work")
                        nc.tensor.matmul(
                            psW[:], lhsT=w2_sb[:, fh * P:(fh + 1) * P],
                            rhs=rhs_sb[:, sl], start=True, stop=True,
                        )
                        # relu + per-partition bias b2[fh*128 + p]; fh halves
                        # split across Activation and DVE to balance engines
                        psWv = psW[:].rearrange(
                            "p (s n b) -> p n s b", s=2, n=NB_H, b=B_LOC)
                        outv = h2v[:, :, 2 * qq:2 * qq + 2, :]
                        if fh == 0:
                            nc.scalar.activation(outv, psWv, AF.Relu,
                                                 bias=b2a[:, fh:fh + 1])
                        else:
                            nc.vector.tensor_scalar(
                                out=outv, in0=psWv,
                                scalar1=b2a[:, fh:fh + 1], scalar2=zcol[:, 0:1],
                                op0=OP.add, op1=OP.max,
                            )
                    h2.append(h2sb)
                # head contraction: hw slab stationary, h2 batch-cols stream;
                # ps_hd accumulates [76, 32b] directly (no final transpose)
                nbh_eff = 1 if "head_1mm" in exp else NB_H
                for i in range(nbh_eff):
                    for fh in range(2):
                        jj = 2 * i + fh
                        last = (nb == nblocks - 1) and (i == nbh_eff - 1) and (fh == 1)
                        nc.tensor.matmul(
                            ps_hd[:],
                            lhsT=hw_sb[:, jj * HW_W:(jj + 1) * HW_W],
                            rhs=h2[fh][:, i * B:(i + 1) * B],
                            start=head_state["first"] or "head_1mm" in exp,
                            stop=last or "head_1mm" in exp,
                            skip_group_check=True,
                        )
                        head_state["first"] = False

            interleave = bool(phases[1] and phases[2] and len(CH_SCHED) == 4)

            if "coll_only" in exp:
                reps = 8 if "coll_x8" in exp else 1
                for _ in range(reps):
                    for c in range(len(CH_SCHED)):
                        a2a_chunk(c)

            # ---- layer 2 message passing, S-stationary: agg2[n, (b f)]
            # accumulates dst-major in ONE psum bank (1 matmul per chunk);
            # 4 PE transposes per tile flip to feature-major for the A2A.
            # quarter-major order: all of quarter q's tiles finish together,
            # releasing A2A chunk q while mp2 continues on quarter q+1.
            # Head blocks for chunk q interleave one quarter later.
            if phases[1]:
                order = [4 * k + q for q in range(4) for k in range(8)]
                if not phases[0]:
                    for t in order:
                        build_s(t)
                # prefetch the src-first-half gathers of quarter-0 tiles: they
                # depend only on H1 rows [0, N/2), so they run during mp1's
                # second half and quarter 0 releases its AllToAll sooner.
                pre_msgs = {}
                if phases[0]:
                    for t in order[:8]:
                        nA = TILE_SPLITS.get(t, 0)
                        if not nA:
                            continue
                        mA = sb_msgA.tile([P, nA * BFH], bf16, tag="msgA",
                                          name=f"msgA{t}")
                        gather(mA[:].rearrange("p (c e) -> p c e", e=BFH),
                               h1_dram[0:N // 2, :], tile_chunks[t][0], nA, BFH,
                               q=t)
                        pre_msgs[t] = (mA, nA)
                for oi, t in enumerate(order):
                    cs = tile_chunks[t]
                    nch0 = len(cs)
                    nreg = nch0 - 1                      # last chunk = self loops
                    c0 = cs[0]
                    if t in pre_msgs:
                        mA, nA = pre_msgs.pop(t)
                        parts = [(mA, i) for i in range(nA)]
                        nB = nreg - nA
                        msg = sb_msg.tile([P, (nB + 1) * BFH], bf16, tag="msg")
                        if nB:
                            gather(msg[:, :nB * BFH].rearrange(
                                       "p (c e) -> p c e", e=BFH),
                                   h1_dram[:, :], c0 + nA, nB, BFH,
                                   seq="mp2_seqgather" in exp, q=t)
                        parts += [(msg, i) for i in range(nB + 1)]
                        self_slot = nB
                    else:
                        msg = sb_msg.tile([P, nch0 * BFH], bf16, tag="msg")
                        if nreg:
                            gather(msg[:, :nreg * BFH].rearrange(
                                       "p (c e) -> p c e", e=BFH),
                                   h1_dram[:, :], c0, nreg, BFH,
                                   seq="mp2_seqgather" in exp, q=t)
                        parts = [(msg, i) for i in range(nch0)]
                        self_slot = nreg
                    # self-loop chunk: the tile's own H1 rows, contiguous
                    nc.sync.dma_start(
                        msg[:, self_slot * BFH:(self_slot + 1) * BFH],
                        h1_dram[t * P:(t + 1) * P, :])
                    agg2 = ps_a2.tile([P, BFH], f32, tag="agg2")
                    cs_eff = cs[:1] if "mp2_1mm" in exp else cs
                    for i, c in enumerate(cs_eff):
                        mt, li = parts[i]
                        nc.tensor.matmul(
                            agg2[:],
                            lhsT=s_all[:, c * P:(c + 1) * P],
                            rhs=mt[:, li * BFH:(li + 1) * BFH],
                            start=(i == 0),
                            stop=(i == len(cs_eff) - 1),
                        )
                    # PSUM -> SBUF (bf16, DVE: the Activation queue carries
                    # head-phase rhs DMAs that wait on collectives and must
                    # not gate mp2), then ship dst-major [n, (b f)] directly
                    agg2_sb = sb.tile([P, BFH], bf16, tag="agg2sb")
                    nc.vector.tensor_copy(agg2_sb[:], agg2[:])
                    k, q = t // 4, t % 4
                    cc, qq = q_to_chunk[q]
                    nc.sync.dma_start(
                        a2a_in[cc][k, qq, :, :, :],
                        agg2_sb[:].rearrange("n (b f) -> n b f", f=F1))
                    if oi % 8 == 7:
                        if qq == CH_SCHED[cc][1] - 1:
                            a2a_chunk(cc)
                        if interleave and q >= 1:
                            head_block(2 * (q - 1))
                            head_block(2 * (q - 1) + 1)

            if interleave:
                for nb in (6, 7):
                    head_block(nb)
            else:
                for nb in range(nblocks):
                    head_block(nb, fire_a2a=not phases[1])

            # head partials land [76, 32b] in PSUM; copy and AllReduce
            part_sb = sb.tile([HW_W, B], f32, tag="part")
            nc.vector.tensor_copy(part_sb[:], ps_hd[:])
            nc.sync.dma_start(ar_in[:, :], part_sb[:])

            # ================= AllReduce partials =================
            if collectives and not ar_local and ar_mode == "ag":
                # AllGather (shared-output fast path) + local 8-way sum
                ag_out = dram.tile([NC_CORES, HW_W, B], f32,
                                   addr_space="Shared")
                nc.gpsimd.collective_compute(
                    "AllGather",
                    mybir.AluOpType.bypass,
                    replica_groups=[list(range(NC_CORES))],
                    ins=[ar_in[:].opt()],
                    outs=[ag_out[:].opt()],
                )
                all_sb = sb.tile([HW_W, NC_CORES * B], f32, tag="allp")
                nc.sync.dma_start(
                    all_sb[:].rearrange("h (s b) -> h s b", s=NC_CORES),
                    ag_out[:].rearrange("s h b -> h s b"))
                red_sb = sb.tile([HW_W, B], f32, tag="red")
                nc.vector.tensor_add(red_sb[:], all_sb[:, 0:B], all_sb[:, B:2 * B])
                for s in range(2, NC_CORES):
                    nc.vector.tensor_add(red_sb[:], red_sb[:],
                                         all_sb[:, s * B:(s + 1) * B])
            else:
                if collectives and not ar_local:
                    nc.gpsimd.collective_compute(
                        "AllReduce",
                        mybir.AluOpType.add,
                        replica_groups=[list(range(NC_CORES))],
                        ins=[ar_in[:].opt()],
                        outs=[ar_out[:].opt()],
                    )
                else:
                    nc.sync.dma_start(ar_out[:, :], ar_in[:, :])
                red_sb = sb.tile([HW_W, B], f32, tag="red")
                nc.sync.dma_start(red_sb[:], ar_out[:, :])

            # ================= final MLP + dueling combine =================
            adv_sb = sb.tile([12, B], f32, tag="adv")
            nc.scalar.activation(adv_sb[:], red_sb[64:76, :], AF.Relu, bias=advb_sb[:])
            # val path
            v1_sb = sb.tile([64, B], f32, tag="v1")
            nc.scalar.activation(v1_sb[:], red_sb[0:64, :], AF.Relu, bias=v1b_sb[:])
            psV = ps_t.tile([64, B], f32, tag="work")
            nc.tensor.matmul(psV[:], lhsT=v2w_sb[:], rhs=v1_sb[:], start=True, stop=True)
            v2_sb = sb.tile([64, B], f32, tag="v2")
            nc.scalar.activation(v2_sb[:], psV[:], AF.Relu, bias=v2b_sb[:])
            psV3 = ps_t.tile([1, B], f32, tag="work")
            nc.tensor.matmul(psV3[:], lhsT=v3w_sb[:], rhs=v2_sb[:], start=True, stop=True)
            val_sb = sb.tile([1, B], f32, tag="val")
            nc.vector.tensor_scalar_add(val_sb[:], psV3[:], v3b_sb[0:1, 0:1])
            # out = cmat.T @ adv + 1.T @ val
            psO = ps_t.tile([12, B], f32, tag="work")
            nc.tensor.matmul(psO[:], lhsT=cmat_sb[:], rhs=adv_sb[:], start=True, stop=False)
            nc.tensor.matmul(psO[:], lhsT=ones1[:, 0:12], rhs=val_sb[:], start=False, stop=True)
            out_sb = sb.tile([12, B], f32, tag="out")
            nc.vector.tensor_copy(out_sb[:], psO[:])
            nc.sync.dma_start(io["out"][:, :], out_sb[:])


# ---------------- driver ----------------

LAST_RESULTS = None

def _input_specs(shared, per_core):
    """name -> (shape, np dtype); per-core entries use per_core[0] shapes."""
    specs = {}
    for k, v in shared.items():
        specs[k] = v
    for k, v in per_core[0].items():
        specs[k] = v
    return specs


def kernel(**inputs) -> np.ndarray:
    import concourse.bacc as bacc
    import concourse.mybir as mybir
    import concourse.tile as tile
    from concourse import bass_utils

    shared, per_core, chunk_tile, nchunk = _prep_host(inputs)

    nc = bacc.Bacc("TRN2", target_bir_lowering=False, debug=False,
                   enable_asserts=False, num_devices=NC_CORES,
                   num_swdge_queues=4)

    io = {}
    specs = _input_specs(shared, per_core)
    for name, arr in specs.items():
        io[name] = nc.dram_tensor(
            name, list(arr.shape), mybir.dt.from_np(arr.dtype), kind="ExternalInput"
        ).ap()
    io["out"] = nc.dram_tensor(
        "out", [12, B], mybir.dt.float32, kind="ExternalOutput"
    ).ap()

    with tile.TileContext(nc) as tc:
        build_program(nc, tc, chunk_tile, nchunk, io)
    nc.compile()

    in_maps = []
    for j in range(NC_CORES):
        m = dict(shared)
        m.update(per_core[j])
        in_maps.append(m)

    def run():
        return bass_utils.run_bass_kernel_spmd(
            nc, in_maps, core_ids=list(range(NC_CORES)),
        )

    def healthy(r):
        # all 8 cores hold the same AllReduced output; divergence or
        # non-finite values indicate a transient device fault -> retry
        outs = [np.asarray(r.results[j]["out"], np.float32)
                for j in range(NC_CORES)]
        if not all(np.isfinite(o).all() for o in outs):
            return False
        return max(float(np.abs(o - outs[0]).max()) for o in outs) <= 1e-3

    res = run()
    if not healthy(res):
        res = run()
    global LAST_RESULTS
    LAST_RESULTS = res
    out = res.results[0]["out"]                      # [12, 32]
    return out.T.reshape(B, 3, 4).copy().astype(np.float32)


if __name__ == "__main__":
    rng = np.random.default_rng(0)
    ei = rng.integers(0, N, (2, E)).astype(np.int64)
    demo = {
        "x": rng.standard_normal((B, N, F_IN), np.float32),
        "edge_index": ei,
        "edge_weight": rng.random(E, np.float32),
        "W1": rng.standard_normal((F_IN, F1), np.float32) / 4,
        "b1": np.zeros(F1, np.float32),
        "W2": rng.standard_normal((F1, F2), np.float32) / 11.3,
        "b2": np.zeros(F2, np.float32),
        "advW": rng.standard_normal((N * F2, 12), np.float32) / 1024,
        "advb": np.zeros(12, np.float32),
        "v1W": rng.standard_normal((N * F2, 64), np.float32) / 1024,
        "v1b": np.zeros(64, np.float32),
        "v2W": rng.standard_normal((64, 64), np.float32) / 8,
        "v2b": np.zeros(64, np.float32),
        "v3W": rng.standard_normal((64, 1), np.float32) / 8,
        "v3b": np.zeros(1, np.float32),
    }
    print(kernel(**demo).shape)

